# revision 3
# baseline (speedup 1.0000x reference)
"""Trainium2 Bass kernel for nn_FSMNSeleNetV3 (FSMN stack + channel maxpool + decoder).

Self-contained: hardcodes all shapes from the problem spec and only imports
numpy + the concourse stack from /opt/trn_rl_repo.

Sharding: pure data parallel over batch. Each of the 8 cores processes 4
batches x 4 channels = 16 independent sequences of T=2048 tokens.

Layout: activations live as [feature_dim, time] in SBUF (features on
partitions). The host pre-transposes x to [B, C, F, T] so the DMA loads are
plain 2D loads. 64-channel tensors (shrink/FSMN stream) pack the two T/2
halves of a sequence onto 128 partitions so every engine sees full-width
tiles; the FSMN conv runs as 11 fused scalar_tensor_tensor taps along the
free (time) dim with a 10-column halo handling the half/sequence boundaries.
"""

import sys

sys.path.insert(0, "/opt/trn_rl_repo")
from contextlib import ExitStack

import numpy as np

import concourse.bass as bass  # noqa: F401  (bass types used via tile/bacc)
import concourse.mybir as mybir
import concourse.tile as tile
from concourse import bacc
from concourse.bass_utils import run_bass_kernel_spmd

F32 = mybir.dt.float32
AF = mybir.ActivationFunctionType
OP = mybir.AluOpType

NCORES = 8
B, T, C, F = 32, 2048, 4, 120
DL, DP, L, LO, RO, S = 128, 64, 5, 10, 1, 5
BPC = B // NCORES  # batches per core
SEQ = BPC * C  # sequences per core
H = T // 2  # half-sequence length (stacked on partitions)
HALO_L = LO - 1  # 9 left halo columns
HW = H + HALO_L + RO  # h buffer width: 1034
NW = T // 512  # 512-token matmul windows per sequence


def build_nc():
    nc = bacc.Bacc("TRN2", target_bir_lowering=False, debug=False, num_devices=NCORES)

    xt_d = nc.dram_tensor("xt", [SEQ, F, T], F32, kind="ExternalInput")
    we0_d = nc.dram_tensor("we0", [F, DL], F32, kind="ExternalInput")
    wedup_d = nc.dram_tensor("wedup", [L, 2 * DP, DL], F32, kind="ExternalInput")
    ws_d = nc.dram_tensor("ws", [L, DL, DP], F32, kind="ExternalInput")
    wd_d = nc.dram_tensor("wd", [DL, S], F32, kind="ExternalInput")
    biases_d = nc.dram_tensor("biases", [DL, L + 1], F32, kind="ExternalInput")
    taps_d = nc.dram_tensor("taps", [2 * DP, L * 11], F32, kind="ExternalInput")
    bd_d = nc.dram_tensor("bd", [S, 1], F32, kind="ExternalInput")
    out_d = nc.dram_tensor("out", [BPC, S, T], F32, kind="ExternalOutput")

    with tile.TileContext(nc) as tc, ExitStack() as ctx:
        wp = ctx.enter_context(tc.tile_pool(name="weights", bufs=1))
        xp = ctx.enter_context(tc.tile_pool(name="x", bufs=2))
        ep = ctx.enter_context(tc.tile_pool(name="e", bufs=2))
        hp = ctx.enter_context(tc.tile_pool(name="h", bufs=2))
        op_ = ctx.enter_context(tc.tile_pool(name="o", bufs=2))
        fp = ctx.enter_context(tc.tile_pool(name="f", bufs=5))
        pp = ctx.enter_context(tc.tile_pool(name="pooled", bufs=2))
        osb = ctx.enter_context(tc.tile_pool(name="osb", bufs=2))
        pse = ctx.enter_context(tc.tile_pool(name="pse", bufs=4, space="PSUM"))
        psh = ctx.enter_context(tc.tile_pool(name="psh", bufs=2, space="PSUM"))
        psd = ctx.enter_context(tc.tile_pool(name="psd", bufs=2, space="PSUM"))

        # --- weights / constants (loaded once) ---
        we0_sb = wp.tile([F, DL], F32)
        nc.sync.dma_start(out=we0_sb[:], in_=we0_d[:])
        wedup_sb = wp.tile([2 * DP, L * DL], F32)
        ws_sb = wp.tile([DL, L * DP], F32)
        for l in range(L):
            nc.sync.dma_start(
                out=wedup_sb[:, l * DL : (l + 1) * DL], in_=wedup_d[l]
            )
            nc.sync.dma_start(out=ws_sb[:, l * DP : (l + 1) * DP], in_=ws_d[l])
        wd_sb = wp.tile([DL, S], F32)
        nc.sync.dma_start(out=wd_sb[:], in_=wd_d[:])
        bias_sb = wp.tile([DL, L + 1], F32)
        nc.sync.dma_start(out=bias_sb[:], in_=biases_d[:])
        taps_sb = wp.tile([2 * DP, L * 11], F32)
        nc.sync.dma_start(out=taps_sb[:], in_=taps_d[:])
        bd_sb = wp.tile([S, 1], F32)
        nc.sync.dma_start(out=bd_sb[:], in_=bd_d[:])

        def tap(l, j):
            return taps_sb[:, l * 11 + j : l * 11 + j + 1]

        for b in range(BPC):
            f_tiles = []
            for c in range(C):
                seq = b * C + c

                x_sb = xp.tile([F, T], F32)
                nc.sync.dma_start(out=x_sb[:], in_=xt_d[seq])

                # ---- unit-0 expand: relu(x @ We0 + be0), K=120 ----
                e_sb = ep.tile([DL, T], F32)
                for w in range(NW):
                    ps = pse.tile([DL, 512], F32)
                    nc.tensor.matmul(
                        ps[:],
                        we0_sb[:],
                        x_sb[:, w * 512 : (w + 1) * 512],
                        start=True,
                        stop=True,
                    )
                    nc.scalar.activation(
                        e_sb[:, w * 512 : (w + 1) * 512],
                        ps[:],
                        AF.Relu,
                        bias=bias_sb[:, 0:1],
                        scale=1.0,
                    )

                o_prev = None
                for l in range(L):
                    if l > 0:
                        # ---- expand l: relu(o @ We[l] + be[l]), K=64, both
                        # halves row-tiled concurrently on the PE ----
                        e_sb = ep.tile([DL, T], F32)
                        for half in range(2):
                            lhsT = wedup_sb[
                                half * DP : (half + 1) * DP, (l - 1) * DL : l * DL
                            ]
                            for w in range(2):
                                ps = pse.tile([DL, 512], F32)
                                rhs = o_prev[
                                    half * DP : (half + 1) * DP,
                                    w * 512 : (w + 1) * 512,
                                ]
                                nc.tensor.matmul(
                                    ps[:],
                                    lhsT,
                                    rhs,
                                    start=True,
                                    stop=True,
                                    tile_position=(half * DP, 0),
                                )
                                col = (half * 2 + w) * 512
                                nc.scalar.activation(
                                    e_sb[:, col : col + 512],
                                    ps[:],
                                    AF.Relu,
                                    bias=bias_sb[:, l : l + 1],
                                    scale=1.0,
                                )

                    # ---- shrink l: h = e @ Ws[l], halves stacked into one
                    # PSUM bank via col tiling ----
                    h_sb = hp.tile([2 * DP, HW], F32)
                    ws_l = ws_sb[:, l * DP : (l + 1) * DP]
                    for w in range(2):
                        ps = psh.tile([2 * DP, 512], F32)
                        nc.tensor.matmul(
                            ps[0:DP, :],
                            ws_l,
                            e_sb[:, w * 512 : (w + 1) * 512],
                            start=True,
                            stop=True,
                            tile_position=(0, 0),
                        )
                        nc.tensor.matmul(
                            ps[DP : 2 * DP, :],
                            ws_l,
                            e_sb[:, H + w * 512 : H + (w + 1) * 512],
                            start=True,
                            stop=True,
                            tile_position=(0, DP),
                        )
                        nc.vector.tensor_copy(
                            h_sb[:, HALO_L + w * 512 : HALO_L + (w + 1) * 512], ps[:]
                        )

                    # ---- halo columns for the conv ----
                    # A-half left edge: t<0 is zero padding
                    nc.gpsimd.memset(h_sb[0:DP, 0:HALO_L], 0.0)
                    # B-half right edge: t=T is zero padding
                    nc.gpsimd.memset(h_sb[DP : 2 * DP, H + HALO_L : HW], 0.0)
                    # B-half left halo = tail of A half
                    nc.vector.tensor_copy(
                        h_sb[DP : 2 * DP, 0:HALO_L], h_sb[0:DP, H : H + HALO_L]
                    )
                    # A-half right halo = head of B half
                    nc.vector.tensor_copy(
                        h_sb[0:DP, H + HALO_L : HW],
                        h_sb[DP : 2 * DP, HALO_L : HALO_L + 1],
                    )

                    # ---- FSMN conv: o = [o_prev +] sum_d c_d * h[t+d] ----
                    o_new = op_.tile([2 * DP, H], F32)
                    core = h_sb[:, HALO_L : HALO_L + H]
                    if l == 0:
                        nc.vector.tensor_scalar_mul(o_new[:], core, tap(l, 9))
                    else:
                        nc.vector.scalar_tensor_tensor(
                            o_new[:], core, tap(l, 9), o_prev[:], OP.mult, OP.add
                        )
                    for j in list(range(9)) + [10]:
                        d = j - 9  # tap offset in time
                        nc.vector.scalar_tensor_tensor(
                            o_new[:],
                            h_sb[:, HALO_L + d : HALO_L + d + H],
                            tap(l, j),
                            o_new[:],
                            OP.mult,
                            OP.add,
                        )
                    o_prev = o_new

                # ---- final expand: relu(o @ We2 + be2) ----
                f_sb = fp.tile([DL, T], F32)
                for half in range(2):
                    lhsT = wedup_sb[half * DP : (half + 1) * DP, 4 * DL : 5 * DL]
                    for w in range(2):
                        ps = pse.tile([DL, 512], F32)
                        rhs = o_prev[
                            half * DP : (half + 1) * DP, w * 512 : (w + 1) * 512
                        ]
                        nc.tensor.matmul(
                            ps[:],
                            lhsT,
                            rhs,
                            start=True,
                            stop=True,
                            tile_position=(half * DP, 0),
                        )
                        col = (half * 2 + w) * 512
                        nc.scalar.activation(
                            f_sb[:, col : col + 512],
                            ps[:],
                            AF.Relu,
                            bias=bias_sb[:, L : L + 1],
                            scale=1.0,
                        )
                f_tiles.append(f_sb)

            # ---- channel maxpool + decoder for this batch ----
            pooled = pp.tile([DL, T], F32)
            nc.vector.tensor_max(pooled[:], f_tiles[0][:], f_tiles[1][:])
            nc.vector.tensor_max(pooled[:], pooled[:], f_tiles[2][:])
            nc.vector.tensor_max(pooled[:], pooled[:], f_tiles[3][:])

            out_sb = osb.tile([S, T], F32)
            for w in range(NW):
                ps = psd.tile([S, 512], F32)
                nc.tensor.matmul(
                    ps[:],
                    wd_sb[:],
                    pooled[:, w * 512 : (w + 1) * 512],
                    start=True,
                    stop=True,
                )
                nc.vector.tensor_scalar_add(
                    out_sb[:, w * 512 : (w + 1) * 512], ps[:], bd_sb[:, 0:1]
                )
            nc.sync.dma_start(out=out_d[b], in_=out_sb[:])

    nc.compile()
    return nc


_NC = None


def get_nc():
    global _NC
    if _NC is None:
        _NC = build_nc()
    return _NC


def prep_in_maps(x, We0, be0, Ws0, wl0, wr0, We, be, Ws, wl, wr, We2, be2, Wd, bd):
    xt = np.ascontiguousarray(x.transpose(0, 2, 3, 1), dtype=np.float32)  # [B,C,F,T]

    wedup = np.stack(
        [np.concatenate([w, w], axis=0) for w in [We[0], We[1], We[2], We[3], We2]]
    ).astype(np.float32)  # [L, 128, 128]
    ws_all = np.stack([Ws0, Ws[0], Ws[1], Ws[2], Ws[3]]).astype(np.float32)
    biases = np.stack([be0, be[0], be[1], be[2], be[3], be2], axis=1).astype(
        np.float32
    )  # [128, 6]

    wl_full = np.concatenate([wl0[None], wl], axis=0)  # [L, 10, 64]
    wr_full = np.concatenate([wr0[None], wr], axis=0)  # [L, 1, 64]
    taps64 = np.concatenate([wl_full, wr_full], axis=1).copy()  # [L, 11, 64]
    taps64[:, LO - 1, :] += 1.0  # conv identity term (o = h + left + right)
    taps = np.tile(
        taps64.transpose(2, 0, 1).reshape(DP, L * 11), (2, 1)
    )  # [128, 55], col = l*11 + j
    taps = np.ascontiguousarray(taps, dtype=np.float32)

    shared = dict(
        we0=np.ascontiguousarray(We0, dtype=np.float32),
        wedup=wedup,
        ws=ws_all,
        wd=np.ascontiguousarray(Wd, dtype=np.float32),
        biases=np.ascontiguousarray(biases),
        taps=taps,
        bd=np.ascontiguousarray(bd.reshape(S, 1), dtype=np.float32),
    )
    in_maps = []
    for k in range(NCORES):
        xs = xt[k * BPC : (k + 1) * BPC].reshape(SEQ, F, T)
        m = dict(shared)
        m["xt"] = np.ascontiguousarray(xs)
        in_maps.append(m)
    return in_maps


def postprocess(results):
    full = np.concatenate([r["out"] for r in results], axis=0)  # [B, S, T]
    return np.ascontiguousarray(full.transpose(0, 2, 1))  # [B, T, S]


def kernel(**inputs):
    nc = get_nc()
    in_maps = prep_in_maps(**inputs)
    res = run_bass_kernel_spmd(nc, in_maps, core_ids=list(range(NCORES)))
    return postprocess(res.results)
